# revision 1
# baseline (speedup 1.0000x reference)
"""4-bit column-block-quantized linear (ColBlockQuantizedLinear) on 8 TRN2 cores.

Math:  out[b,o] = scales[o] * (sum_i inp[b,i]*wq[o,i] - zeros[o]*rowsum[b])
where wq comes from packed bytes q[o,j] (j = i//2): even i -> low nibble,
odd i -> high nibble.

Device-side identity (all O(O*I) work stays on-device):
    sum_j l*a + sum_j h*b = sum_j q*a + sum_j h*(b-16a)
with q = 16h + l, a[j]=inp[:,2j], b[j]=inp[:,2j+1].

The h-stream never materializes h as an integer: a 4-instruction uint16
bit-trick on DVE writes the bf16 BIT PATTERN 0x4300|(h<<3) (= value 128+8h,
linear in h) at 4x DVE mode.  The matmul pairs it with c' = (b-16a)/8 and the
constant 128*sum(c') falls out as a rank-1 correction row.  The q-stream is a
plain u8->bf16 cast (exact, 0..255) split across ACT and GPSIMD.  Activations
are hi/lo bf16-split so the bf16 matmuls give ~fp32 accuracy; zeros*rowsum and
the 128-offset are a K=6 correction matmul with hi/lo-split factors.

Host byte layout: per core the packed bytes [2048, 1376] are column-paired as
(m, 688+m) into uint16 [2048, 688], so the bit-trick's two output streams land
contiguously in natural column order; the q-cast output is column-interleaved
and its matmuls read it through stride-2 APs.

Sharding: column-parallel over out_features (1376 rows/core), inputs
replicated; per-core output [16,1376] gathered on host.
"""

import numpy as np
import ml_dtypes

B = 16
I = 4096
O = 11008
NCORES = 8
OS = O // NCORES          # 1376 out-features per core
HOS = OS // 2             # 688, u16-packed column count
HALF = I // 2             # 2048 packed columns
KT = HALF // 128          # 16 contraction tiles
# psum-bank o-blocks, each a single arithmetic progression in the interleaved
# q-cast layout (no block crosses the 688-column half boundary)
BLKS = [(0, 512), (512, 176), (688, 512), (1200, 176)]
N_ACT_CAST = 10           # q-cast tiles on ACT; rest on GPSIMD

BF16 = ml_dtypes.bfloat16

_CACHE = {}


def _split_hi_lo(x64):
    """Split float64 array into (hi, lo) bf16 parts: hi+lo ~= x to ~2^-17."""
    hi = x64.astype(BF16)
    lo = (x64 - hi.astype(np.float64)).astype(BF16)
    return hi, lo


def _qcast_ap(qb, s, n):
    """Stride-2 AP over the interleaved q-cast tile covering natural columns
    [s, s+n) (s,n within one half)."""
    if s < HOS:
        return qb[:, 2 * s : 2 * (s + n) : 2]
    return qb[:, 2 * (s - HOS) + 1 : 2 * (s - HOS + n) : 2]


def _build_program():
    import concourse.bacc as bacc
    import concourse.mybir as mybir
    import concourse.tile as tile

    dt = mybir.dt
    op = mybir.AluOpType
    nc = bacc.Bacc("TRN2", target_bir_lowering=False)

    q = nc.dram_tensor("q", [HALF, HOS], dt.uint16, kind="ExternalInput")
    statA = nc.dram_tensor("statA", [128, KT * 64], dt.bfloat16, kind="ExternalInput")
    statC = nc.dram_tensor("statC", [128, KT * 64], dt.bfloat16, kind="ExternalInput")
    corrL = nc.dram_tensor("corrL", [6, 64], dt.bfloat16, kind="ExternalInput")
    corrR = nc.dram_tensor("corrR", [6, OS], dt.bfloat16, kind="ExternalInput")
    sc = nc.dram_tensor("sc", [B, OS], dt.float32, kind="ExternalInput")
    out = nc.dram_tensor("out", [B, OS], dt.float32, kind="ExternalOutput")

    with tile.TileContext(nc) as tc:
        with (
            tc.tile_pool(name="consts", bufs=1) as cpool,
            tc.tile_pool(name="qp", bufs=3) as qpool,
            tc.tile_pool(name="tp", bufs=2) as tpool,
            tc.tile_pool(name="wp", bufs=3) as wpool,
            tc.tile_pool(name="op", bufs=2) as opool,
            tc.tile_pool(name="ps", bufs=1, space="PSUM") as pspool,
        ):
            statA_sb = cpool.tile([128, KT * 64], dt.bfloat16, name="statA_sb")
            statC_sb = cpool.tile([128, KT * 64], dt.bfloat16, name="statC_sb")
            corrL_sb = cpool.tile([6, 64], dt.bfloat16, name="corrL_sb")
            corrR_sb = cpool.tile([6, OS], dt.bfloat16, name="corrR_sb")
            sc_sb = cpool.tile([B, OS], dt.float32, name="sc_sb")
            nc.sync.dma_start(statA_sb, statA[:, :])
            nc.sync.dma_start(statC_sb, statC[:, :])
            nc.sync.dma_start(corrL_sb, corrL[:, :])
            nc.sync.dma_start(corrR_sb, corrR[:, :])
            nc.sync.dma_start(sc_sb, sc[:, :])

            psums = [
                pspool.tile([64, n], dt.float32, name=f"ps{i}")
                for i, (s, n) in enumerate(BLKS)
            ]

            for kt in range(KT):
                qt = qpool.tile([128, HOS], dt.uint16, name="qt", tag="qt")
                nc.sync.dma_start(qt, q[kt * 128 : (kt + 1) * 128, :])
                qb = wpool.tile([128, OS], dt.bfloat16, name="qb", tag="qb")
                hb = wpool.tile([128, OS], dt.bfloat16, name="hb", tag="hb")
                hbu = hb.bitcast(dt.uint16)
                t1 = tpool.tile([128, HOS], dt.uint16, name="t1", tag="t1")
                t2 = tpool.tile([128, HOS], dt.uint16, name="t2", tag="t2")
                # q-cast (exact bf16 of 0..255); interleaved column order
                if kt < N_ACT_CAST:
                    nc.scalar.activation(
                        qb, qt.bitcast(dt.uint8), mybir.ActivationFunctionType.Copy
                    )
                else:
                    nc.gpsimd.tensor_copy(qb, qt.bitcast(dt.uint8))
                # h-stream bit trick: bf16 bits 0x4300|(h<<3) = 128+8h
                nc.vector.tensor_scalar(t1, qt, 1, None, op.logical_shift_right)
                nc.vector.tensor_scalar(
                    hbu[:, 0:HOS], t1, 0x78, 0x4300, op.bitwise_and, op.bitwise_or
                )
                nc.vector.tensor_scalar(
                    t2, t1, 8, 0x78, op.logical_shift_right, op.bitwise_and
                )
                nc.vector.tensor_scalar(
                    hbu[:, HOS:OS], t2, 0x4300, None, op.bitwise_or
                )
                for i, (s, n) in enumerate(BLKS):
                    nc.tensor.matmul(
                        psums[i],
                        statA_sb[:, kt * 64 : kt * 64 + 64],
                        _qcast_ap(qb, s, n),
                        start=(kt == 0),
                        stop=False,
                    )
                    nc.tensor.matmul(
                        psums[i],
                        statC_sb[:, kt * 64 : kt * 64 + 64],
                        hb[:, s : s + n],
                        start=False,
                        stop=False,
                    )

            for i, (s, n) in enumerate(BLKS):
                # rank-1 corrections: -zeros*rowsum and -128*sum(c')
                nc.tensor.matmul(
                    psums[i],
                    corrL_sb,
                    corrR_sb[:, s : s + n],
                    start=False,
                    stop=True,
                )
                t0 = opool.tile([B, n], dt.float32, name="t0", tag=f"t0{i}")
                t = opool.tile([B, n], dt.float32, name="t", tag=f"t{i}")
                o = opool.tile([B, n], dt.float32, name="o", tag=f"o{i}")
                # lo-group psum -> sbuf on ACT (only one psum read allowed per TT)
                nc.scalar.activation(
                    t0, psums[i][32:48, :], mybir.ActivationFunctionType.Copy
                )
                nc.vector.tensor_tensor(t, psums[i][0:16, :], t0, op.add)
                nc.vector.tensor_tensor(o, t, sc_sb[:, s : s + n], op.mult)
                nc.sync.dma_start(out[:, s : s + n], o)

    nc.finalize()
    return nc


def _get_program():
    if "nc" not in _CACHE:
        _CACHE["nc"] = _build_program()
    return _CACHE["nc"]


def _host_prep(inp, quant_weight, scales, zeros):
    """Build per-core input maps (layout/precision prep only, no dequant math)."""
    inp64 = np.asarray(inp, dtype=np.float64)
    a = inp64[:, 0::2].T.copy()  # [HALF, B] even-i activations (pair with l)
    b = inp64[:, 1::2].T.copy()  # [HALF, B] odd-i activations (pair with h)
    # q-stream pairs with a; bit-trick h-stream pairs with c' = (b-16a)/8
    cp = (b - 16.0 * a) / 8.0
    a_hi, a_lo = _split_hi_lo(a)
    c_hi, c_lo = _split_hi_lo(cp)

    statA = np.zeros((128, KT * 64), dtype=BF16)
    statC = np.zeros((128, KT * 64), dtype=BF16)
    for kt in range(KT):
        rows = slice(kt * 128, (kt + 1) * 128)
        statA[:, kt * 64 : kt * 64 + 16] = a_hi[rows]
        statA[:, kt * 64 + 32 : kt * 64 + 48] = a_lo[rows]
        statC[:, kt * 64 : kt * 64 + 16] = c_hi[rows]
        statC[:, kt * 64 + 32 : kt * 64 + 48] = c_lo[rows]

    rowsum = inp64.sum(axis=1)  # [B]
    rs_hi, rs_lo = _split_hi_lo(rowsum)
    s_c = cp.sum(axis=0)  # [B]  sum_j c'[j,b]
    sc_hi, sc_lo = _split_hi_lo(s_c)
    corrL = np.zeros((6, 64), dtype=BF16)
    corrL[0, :16] = rs_hi
    corrL[1, :16] = rs_hi
    corrL[2, :16] = rs_lo
    corrL[3, :16] = rs_lo
    corrL[4, :16] = sc_hi
    corrL[5, :16] = sc_lo

    qw = np.asarray(quant_weight)
    scales = np.asarray(scales, dtype=np.float64).reshape(-1)
    zeros = np.asarray(zeros, dtype=np.float64).reshape(-1)

    in_maps = []
    for cidx in range(NCORES):
        rows = slice(cidx * OS, (cidx + 1) * OS)
        qc = qw[rows].astype(np.uint8).T  # [HALF, OS] natural columns
        # byte-pair columns (m, 688+m) -> uint16 elements
        qc2 = np.empty((HALF, OS), dtype=np.uint8)
        qc2[:, 0::2] = qc[:, :HOS]
        qc2[:, 1::2] = qc[:, HOS:]
        qu16 = np.ascontiguousarray(qc2).view(np.uint16)  # [HALF, HOS]
        z = zeros[rows]
        z_hi, z_lo = _split_hi_lo(z)
        corrR = np.zeros((6, OS), dtype=BF16)
        corrR[0] = -z_hi
        corrR[1] = -z_lo
        corrR[2] = -z_hi
        corrR[3] = -z_lo
        corrR[4] = -128.0
        corrR[5] = -128.0
        sc_c = np.broadcast_to(scales[rows].astype(np.float32), (B, OS)).copy()
        in_maps.append(
            {
                "q": qu16,
                "statA": statA,
                "statC": statC,
                "corrL": corrL,
                "corrR": corrR,
                "sc": sc_c,
            }
        )
    return in_maps


def kernel(inp, quant_weight, scales, zeros):
    from concourse.bass_utils import run_bass_kernel_spmd

    nc = _get_program()
    in_maps = _host_prep(inp, quant_weight, scales, zeros)
    res = run_bass_kernel_spmd(nc, in_maps, core_ids=list(range(NCORES)))
    out = np.concatenate(
        [res.results[c]["out"] for c in range(NCORES)], axis=1
    )
    return np.ascontiguousarray(out.astype(np.float32))



# revision 2
# speedup vs baseline: 1.7277x; 1.7277x over previous
"""4-bit column-block-quantized linear (ColBlockQuantizedLinear) on 8 TRN2 cores.

Math:  out[b,o] = scales[o] * (sum_i inp[b,i]*wq[o,i] - zeros[o]*rowsum[b])
with packed bytes q[j,o] (j = i//2): low nibble l = wq[o,2j], high nibble
h = wq[o,2j+1].  Identity: sum_j a_j*l_j + b_j*h_j = sum_j a_j*q_j + c_j*h_j
with c = b - 16a, q = 16h + l.

Device scheme (fp16 bit-trick): the fp16 bit pattern 0x5800|x encodes the
value 128 + x/8 EXACTLY for any 8-bit x.  So each weight stream is ONE
dual-op DVE tensor_scalar pass over the packed u16 data:
    Qlo = (q16 & 0x00FF) | 0x5800   -> 128 + q_lo/8      (pairs with 8a)
    Qhi = (q16 >>  8)    | 0x5800   -> 128 + q_hi/8      (pairs with 8a)
    Hlo = (q16 & 0x00F0) | 0x5800   -> 128 + 2*h_lo      (pairs with c/2)
    Hhi = (q16 >> 12)    | 0x5800   -> 128 + h_hi/8      (pairs with 8c)
The +128 offsets are exact rank-1 terms -128*sum_j(coef_j) folded, together
with -zeros*rowsum, into a K=4 fp32 correction matmul issued FIRST (start=True)
so there is no tail.  Stationary activation factors are single fp16 (no hi/lo
split needed: 2^-11 rounding -> ~5e-4 output rel err).  Scales are applied
on-device by a per-psum-block DVE tensor_tensor multiply.

kt tiles are processed in PAIRS (one DVE pass covers 2 tiles side by side) to
amortize the ~150ns fixed DVE instruction overhead.

Host byte layout: per core packed bytes [2048, 1376] are column-paired as
(m, 688+m) into uint16, then rows regrouped so q_dram[r, kt*688+m] holds
contraction row kt*128+r -- one contiguous DMA per kt pair.

Sharding: column-parallel over out_features (1376 rows/core), inputs
replicated; per-core output [16,1376] gathered on host.
"""

import numpy as np

B = 16
I = 4096
O = 11008
NCORES = 8
OS = O // NCORES          # 1376 out-features per core
HOS = OS // 2             # 688 packed u16 columns
HALF = I // 2             # 2048 packed (contraction) rows
KT = HALF // 128          # 16 contraction tiles
NPAIR = KT // 2           # 8 kt pairs
# psum o-blocks (each within one 688-column half, <=512 cols per fp32 bank)
BLKS = [(0, 512), (512, 176), (688, 512), (1200, 176)]

F16 = np.float16

_CACHE = {}


def _build_program():
    import concourse.bacc as bacc
    import concourse.mybir as mybir
    import concourse.tile as tile

    dt = mybir.dt
    op = mybir.AluOpType
    nc = bacc.Bacc("TRN2", target_bir_lowering=False)

    q = nc.dram_tensor("q", [128, KT * HOS], dt.uint16, kind="ExternalInput")
    stat = nc.dram_tensor("stat", [128, KT * 48], dt.float16, kind="ExternalInput")
    corrL = nc.dram_tensor("corrL", [4, 16], dt.float32, kind="ExternalInput")
    corrR = nc.dram_tensor("corrR", [4, OS], dt.float32, kind="ExternalInput")
    sc = nc.dram_tensor("sc", [B, OS], dt.float32, kind="ExternalInput")
    out = nc.dram_tensor("out", [B, OS], dt.float32, kind="ExternalOutput")

    with tile.TileContext(nc) as tc:
        with (
            tc.tile_pool(name="consts", bufs=1) as cpool,
            tc.tile_pool(name="qp", bufs=3) as qpool,
            tc.tile_pool(name="wp", bufs=2) as wpool,
            tc.tile_pool(name="op", bufs=2) as opool,
            tc.tile_pool(name="ps", bufs=1, space="PSUM") as pspool,
        ):
            stat_sb = cpool.tile([128, KT * 48], dt.float16, name="stat_sb")
            corrL_sb = cpool.tile([4, 16], dt.float32, name="corrL_sb")
            corrR_sb = cpool.tile([4, OS], dt.float32, name="corrR_sb")
            sc_sb = cpool.tile([B, OS], dt.float32, name="sc_sb")
            nc.sync.dma_start(stat_sb, stat[:, :])
            nc.sync.dma_start(corrL_sb, corrL[:, :])
            nc.sync.dma_start(corrR_sb, corrR[:, :])
            nc.sync.dma_start(sc_sb, sc[:, :])

            psums = [
                pspool.tile([B, n], dt.float32, name=f"ps{i}")
                for i, (s, n) in enumerate(BLKS)
            ]

            # corrections first: -128*sum(coef) offsets and -zeros*rowsum
            for i, (s, n) in enumerate(BLKS):
                nc.tensor.matmul(
                    psums[i], corrL_sb, corrR_sb[:, s : s + n],
                    start=True, stop=False,
                )

            for p in range(NPAIR):
                qt = qpool.tile([128, 2 * HOS], dt.uint16, name="qt", tag="qt")
                nc.sync.dma_start(qt, q[:, p * 2 * HOS : (p + 1) * 2 * HOS])
                qlo = wpool.tile([128, 2 * HOS], dt.uint16, name="qlo", tag="qlo")
                qhi = wpool.tile([128, 2 * HOS], dt.uint16, name="qhi", tag="qhi")
                hlo = wpool.tile([128, 2 * HOS], dt.uint16, name="hlo", tag="hlo")
                hhi = wpool.tile([128, 2 * HOS], dt.uint16, name="hhi", tag="hhi")
                nc.vector.tensor_scalar(
                    qlo, qt, 0x00FF, 0x5800, op.bitwise_and, op.bitwise_or
                )
                nc.vector.tensor_scalar(
                    qhi, qt, 8, 0x5800, op.logical_shift_right, op.bitwise_or
                )
                nc.vector.tensor_scalar(
                    hlo, qt, 0x00F0, 0x5800, op.bitwise_and, op.bitwise_or
                )
                nc.vector.tensor_scalar(
                    hhi, qt, 12, 0x5800, op.logical_shift_right, op.bitwise_or
                )
                qlo16 = qlo.bitcast(dt.float16)
                qhi16 = qhi.bitcast(dt.float16)
                hlo16 = hlo.bitcast(dt.float16)
                hhi16 = hhi.bitcast(dt.float16)
                for h in range(2):
                    kt = 2 * p + h
                    last = kt == KT - 1
                    off = h * HOS
                    sq = stat_sb[:, kt * 48 : kt * 48 + 16]
                    shlo = stat_sb[:, kt * 48 + 16 : kt * 48 + 32]
                    shhi = stat_sb[:, kt * 48 + 32 : kt * 48 + 48]
                    for i, (s, n) in enumerate(BLKS):
                        if s < HOS:
                            a, b_ = off + s, off + s + n
                            nc.tensor.matmul(
                                psums[i], sq, qlo16[:, a:b_],
                                start=False, stop=False,
                            )
                            nc.tensor.matmul(
                                psums[i], shlo, hlo16[:, a:b_],
                                start=False, stop=last,
                            )
                        else:
                            a, b_ = off + s - HOS, off + s - HOS + n
                            nc.tensor.matmul(
                                psums[i], sq, qhi16[:, a:b_],
                                start=False, stop=False,
                            )
                            nc.tensor.matmul(
                                psums[i], shhi, hhi16[:, a:b_],
                                start=False, stop=last,
                            )

            for i, (s, n) in enumerate(BLKS):
                o = opool.tile([B, n], dt.float32, name="o", tag=f"o{i}")
                nc.vector.tensor_tensor(
                    o, psums[i], sc_sb[:, s : s + n], op.mult
                )
                nc.sync.dma_start(out[:, s : s + n], o)

    nc.finalize()
    return nc


def _get_program():
    if "nc" not in _CACHE:
        _CACHE["nc"] = _build_program()
    return _CACHE["nc"]


def _host_prep(inp, quant_weight, scales, zeros):
    """Per-core input maps: layout/precision prep only, no O(O*I) math."""
    inp64 = np.asarray(inp, dtype=np.float64)
    a = inp64[:, 0::2].T  # [HALF, B] even-i activations (pair with l / q)
    b = inp64[:, 1::2].T  # [HALF, B] odd-i activations (pair with h)
    c = b - 16.0 * a

    sq = (8.0 * a).astype(F16)      # [HALF, B]
    shlo = (c / 2.0).astype(F16)
    shhi = (8.0 * c).astype(F16)

    stat = np.zeros((128, KT * 48), dtype=F16)
    for kt in range(KT):
        rows = slice(kt * 128, (kt + 1) * 128)
        stat[:, kt * 48 : kt * 48 + 16] = sq[rows]
        stat[:, kt * 48 + 16 : kt * 48 + 32] = shlo[rows]
        stat[:, kt * 48 + 32 : kt * 48 + 48] = shhi[rows]

    # correction batch vectors from the ROUNDED stationaries (exact cancel)
    sum_sq = sq.astype(np.float64).sum(axis=0)      # [B]
    sum_shlo = shlo.astype(np.float64).sum(axis=0)
    sum_shhi = shhi.astype(np.float64).sum(axis=0)
    rowsum = inp64.sum(axis=1)                      # [B]
    corrL = np.zeros((4, 16), dtype=np.float32)
    corrL[0] = sum_sq
    corrL[1] = sum_shlo
    corrL[2] = sum_shhi
    corrL[3] = rowsum

    qw = np.asarray(quant_weight)
    scales = np.asarray(scales, dtype=np.float64).reshape(-1)
    zeros = np.asarray(zeros, dtype=np.float64).reshape(-1)

    in_maps = []
    for cidx in range(NCORES):
        rows = slice(cidx * OS, (cidx + 1) * OS)
        qc = qw[rows].astype(np.uint8).T  # [HALF, OS] natural columns
        # byte-pair columns (m, 688+m) -> uint16 elements
        qc2 = np.empty((HALF, OS), dtype=np.uint8)
        qc2[:, 0::2] = qc[:, :HOS]
        qc2[:, 1::2] = qc[:, HOS:]
        qu16 = np.ascontiguousarray(qc2).view(np.uint16)  # [HALF, HOS]
        # regroup rows: q_dram[r, kt*HOS + m] = qu16[kt*128 + r, m]
        q_dram = np.ascontiguousarray(
            qu16.reshape(KT, 128, HOS).transpose(1, 0, 2).reshape(128, KT * HOS)
        )
        z = zeros[rows]
        corrR = np.zeros((4, OS), dtype=np.float32)
        corrR[0] = -128.0
        corrR[1, :HOS] = -128.0
        corrR[2, HOS:] = -128.0
        corrR[3] = -z
        sc_c = np.broadcast_to(scales[rows].astype(np.float32), (B, OS)).copy()
        in_maps.append(
            {
                "q": q_dram,
                "stat": stat,
                "corrL": corrL,
                "corrR": corrR,
                "sc": sc_c,
            }
        )
    return in_maps


def kernel(inp, quant_weight, scales, zeros):
    from concourse.bass_utils import run_bass_kernel_spmd

    nc = _get_program()
    in_maps = _host_prep(inp, quant_weight, scales, zeros)
    res = run_bass_kernel_spmd(nc, in_maps, core_ids=list(range(NCORES)))
    out = np.concatenate(
        [res.results[c]["out"] for c in range(NCORES)], axis=1
    )
    return np.ascontiguousarray(out.astype(np.float32))


# revision 3
# speedup vs baseline: 1.7288x; 1.0006x over previous
"""4-bit column-block-quantized linear (ColBlockQuantizedLinear) on 8 TRN2 cores.

Math:  out[b,o] = scales[o] * (sum_i inp[b,i]*wq[o,i] - zeros[o]*rowsum[b])
with packed bytes q[j,o] (j = i//2): low nibble l = wq[o,2j], high nibble
h = wq[o,2j+1].  Identity: sum_j a_j*l_j + b_j*h_j = sum_j a_j*q_j + c_j*h_j
with c = b - 16a, q = 16h + l.

Device scheme (fp16 bit-trick): the fp16 bit pattern 0x5800|x encodes the
value 128 + x/8 EXACTLY for any 8-bit x.  So each weight stream is ONE
dual-op DVE tensor_scalar pass over the packed u16 data:
    Qlo = (q16 & 0x00FF) | 0x5800   -> 128 + q_lo/8      (pairs with 8a)
    Qhi = (q16 >>  8)    | 0x5800   -> 128 + q_hi/8      (pairs with 8a)
    Hlo = (q16 & 0x00F0) | 0x5800   -> 128 + 2*h_lo      (pairs with c/2)
    Hhi = (q16 >> 12)    | 0x5800   -> 128 + h_hi/8      (pairs with 8c)
The +128 offsets cancel exactly against rank-1 rows built from the SAME
fp16-rounded stationaries, folded with -zeros*rowsum into a K=9 fp16 hi/lo
correction matmul issued FIRST (start=True) so there is no tail.  Stationary
activation factors are single fp16.  Scales are applied on-device by
per-psum-block DVE tensor_tensor multiplies into one output tile.

A few zero matmuls on memset tiles warm the PE (HAM un-throttle needs
~3.4us of activity) while the first DMAs/DVE passes run.  kt tiles are
processed in chunks [1,1,2,2,...] - small first chunks cut pipeline rampup,
pairs amortize the ~150ns fixed DVE pass overhead.

Host byte layout: per core packed bytes [2048, 1376] are column-paired as
(m, 688+m) into uint16, rows regrouped so q_dram[r, kt*688+m] holds
contraction row kt*128+r -> one contiguous DMA per chunk.

Sharding: column-parallel over out_features (1376 rows/core), inputs
replicated; per-core output [16,1376] gathered on host.
"""

import numpy as np

B = 16
I = 4096
O = 11008
NCORES = 8
OS = O // NCORES          # 1376 out-features per core
HOS = OS // 2             # 688 packed u16 columns
HALF = I // 2             # 2048 packed (contraction) rows
KT = HALF // 128          # 16 contraction tiles
CHUNKS = [1, 1] + [2] * 7  # kt tiles per processing chunk
KC = 9                    # correction matmul contraction size
NDUMMY = 6                # PE warmup matmuls
# psum o-blocks (each within one 688-column half, <=512 cols per fp32 bank)
BLKS = [(0, 512), (512, 176), (688, 512), (1200, 176)]

F16 = np.float16

_CACHE = {}


def _build_program():
    import concourse.bacc as bacc
    import concourse.mybir as mybir
    import concourse.tile as tile

    dt = mybir.dt
    op = mybir.AluOpType
    nc = bacc.Bacc("TRN2", target_bir_lowering=False)

    q = nc.dram_tensor("q", [128, KT * HOS], dt.uint16, kind="ExternalInput")
    stat = nc.dram_tensor("stat", [128, KT * 48], dt.float16, kind="ExternalInput")
    corrL = nc.dram_tensor("corrL", [KC, 16], dt.float16, kind="ExternalInput")
    corrR = nc.dram_tensor("corrR", [KC, OS], dt.float16, kind="ExternalInput")
    sc = nc.dram_tensor("sc", [B, OS], dt.float32, kind="ExternalInput")
    out = nc.dram_tensor("out", [B, OS], dt.float32, kind="ExternalOutput")

    cstart = [0]
    for w in CHUNKS:
        cstart.append(cstart[-1] + w)

    with tile.TileContext(nc) as tc:
        with (
            tc.tile_pool(name="consts", bufs=1) as cpool,
            tc.tile_pool(name="qp", bufs=3) as qpool,
            tc.tile_pool(name="wp", bufs=3) as wpool,
            tc.tile_pool(name="op", bufs=1) as opool,
            tc.tile_pool(name="ps", bufs=1, space="PSUM") as pspool,
        ):
            # PE warmup: zero matmuls while DMAs/DVE fill the pipeline
            dummy = cpool.tile([128, 512], dt.float16, name="dummy")
            ps_w = pspool.tile([16, 512], dt.float32, name="ps_w")
            nc.vector.memset(dummy, 0.0)
            for _ in range(NDUMMY):
                nc.tensor.matmul(
                    ps_w, dummy[:, 0:16], dummy, start=True, stop=True,
                    skip_group_check=True,
                )

            corrL_sb = cpool.tile([KC, 16], dt.float16, name="corrL_sb")
            corrR_sb = cpool.tile([KC, OS], dt.float16, name="corrR_sb")
            stat_sb = cpool.tile([128, KT * 48], dt.float16, name="stat_sb")
            sc_sb = cpool.tile([B, OS], dt.float32, name="sc_sb")
            nc.scalar.dma_start(corrL_sb, corrL[:, :])
            nc.scalar.dma_start(corrR_sb, corrR[:, :])
            nc.scalar.dma_start(stat_sb, stat[:, :])
            nc.scalar.dma_start(sc_sb, sc[:, :])

            psums = [
                pspool.tile([B, n], dt.float32, name=f"ps{i}")
                for i, (s, n) in enumerate(BLKS)
            ]

            # corrections first: -128*sum(coef) offsets and -zeros*rowsum
            for i, (s, n) in enumerate(BLKS):
                nc.tensor.matmul(
                    psums[i], corrL_sb, corrR_sb[:, s : s + n],
                    start=True, stop=False,
                )

            for ci, cw in enumerate(CHUNKS):
                k0, w = cstart[ci], cw * HOS
                qt = qpool.tile([128, w], dt.uint16, name=f"qt{cw}", tag="qt")
                nc.sync.dma_start(qt, q[:, k0 * HOS : k0 * HOS + w])
                qlo = wpool.tile([128, w], dt.uint16, name=f"qlo{cw}", tag="qlo")
                qhi = wpool.tile([128, w], dt.uint16, name=f"qhi{cw}", tag="qhi")
                hlo = wpool.tile([128, w], dt.uint16, name=f"hlo{cw}", tag="hlo")
                hhi = wpool.tile([128, w], dt.uint16, name=f"hhi{cw}", tag="hhi")
                nc.vector.tensor_scalar(
                    qlo, qt, 0x00FF, 0x5800, op.bitwise_and, op.bitwise_or
                )
                nc.vector.tensor_scalar(
                    qhi, qt, 8, 0x5800, op.logical_shift_right, op.bitwise_or
                )
                nc.vector.tensor_scalar(
                    hlo, qt, 0x00F0, 0x5800, op.bitwise_and, op.bitwise_or
                )
                nc.vector.tensor_scalar(
                    hhi, qt, 12, 0x5800, op.logical_shift_right, op.bitwise_or
                )
                qlo16 = qlo.bitcast(dt.float16)
                qhi16 = qhi.bitcast(dt.float16)
                hlo16 = hlo.bitcast(dt.float16)
                hhi16 = hhi.bitcast(dt.float16)
                for h in range(cw):
                    kt = k0 + h
                    last = kt == KT - 1
                    off = h * HOS
                    sq = stat_sb[:, kt * 48 : kt * 48 + 16]
                    shlo = stat_sb[:, kt * 48 + 16 : kt * 48 + 32]
                    shhi = stat_sb[:, kt * 48 + 32 : kt * 48 + 48]
                    for i, (s, n) in enumerate(BLKS):
                        if s < HOS:
                            a, b_ = off + s, off + s + n
                            nc.tensor.matmul(
                                psums[i], sq, qlo16[:, a:b_],
                                start=False, stop=False,
                            )
                            nc.tensor.matmul(
                                psums[i], shlo, hlo16[:, a:b_],
                                start=False, stop=last,
                            )
                        else:
                            a, b_ = off + s - HOS, off + s - HOS + n
                            nc.tensor.matmul(
                                psums[i], sq, qhi16[:, a:b_],
                                start=False, stop=False,
                            )
                            nc.tensor.matmul(
                                psums[i], shhi, hhi16[:, a:b_],
                                start=False, stop=last,
                            )

            o = opool.tile([B, OS], dt.float32, name="o")
            for i, (s, n) in enumerate(BLKS):
                nc.vector.tensor_tensor(
                    o[:, s : s + n], psums[i], sc_sb[:, s : s + n], op.mult
                )
            nc.scalar.dma_start(out[:, :], o)

    nc.finalize()
    return nc


def _get_program():
    if "nc" not in _CACHE:
        _CACHE["nc"] = _build_program()
    return _CACHE["nc"]


def _split_hi_lo(x64):
    hi = x64.astype(F16)
    lo = (x64 - hi.astype(np.float64)).astype(F16)
    return hi, lo


def _host_prep(inp, quant_weight, scales, zeros):
    """Per-core input maps: layout/precision prep only, no O(O*I) math."""
    inp64 = np.asarray(inp, dtype=np.float64)
    a = inp64[:, 0::2].T  # [HALF, B] even-i activations (pair with l / q)
    b = inp64[:, 1::2].T  # [HALF, B] odd-i activations (pair with h)
    c = b - 16.0 * a

    sq = (8.0 * a).astype(F16)      # [HALF, B]
    shlo = (c / 2.0).astype(F16)
    shhi = (8.0 * c).astype(F16)

    stat = np.zeros((128, KT * 48), dtype=F16)
    for kt in range(KT):
        rows = slice(kt * 128, (kt + 1) * 128)
        stat[:, kt * 48 : kt * 48 + 16] = sq[rows]
        stat[:, kt * 48 + 16 : kt * 48 + 32] = shlo[rows]
        stat[:, kt * 48 + 32 : kt * 48 + 48] = shhi[rows]

    # correction batch vectors from the ROUNDED stationaries (exact cancel)
    sum_sq = sq.astype(np.float64).sum(axis=0)      # [B]
    sum_shlo = shlo.astype(np.float64).sum(axis=0)
    sum_shhi = shhi.astype(np.float64).sum(axis=0)
    rowsum = inp64.sum(axis=1)                      # [B]
    sq_h, sq_l = _split_hi_lo(sum_sq)
    slo_h, slo_l = _split_hi_lo(sum_shlo)
    shi_h, shi_l = _split_hi_lo(sum_shhi)
    rs_h, rs_l = _split_hi_lo(rowsum)
    corrL = np.zeros((KC, 16), dtype=F16)
    corrL[0], corrL[1] = sq_h, sq_l
    corrL[2], corrL[3] = slo_h, slo_l
    corrL[4], corrL[5] = shi_h, shi_l
    corrL[6], corrL[7] = rs_h, rs_h
    corrL[8] = rs_l

    qw = np.asarray(quant_weight)
    scales = np.asarray(scales, dtype=np.float64).reshape(-1)
    zeros = np.asarray(zeros, dtype=np.float64).reshape(-1)

    in_maps = []
    for cidx in range(NCORES):
        rows = slice(cidx * OS, (cidx + 1) * OS)
        qc = qw[rows].astype(np.uint8).T  # [HALF, OS] natural columns
        # byte-pair columns (m, 688+m) -> uint16 elements
        qc2 = np.empty((HALF, OS), dtype=np.uint8)
        qc2[:, 0::2] = qc[:, :HOS]
        qc2[:, 1::2] = qc[:, HOS:]
        qu16 = np.ascontiguousarray(qc2).view(np.uint16)  # [HALF, HOS]
        # regroup rows: q_dram[r, kt*HOS + m] = qu16[kt*128 + r, m]
        q_dram = np.ascontiguousarray(
            qu16.reshape(KT, 128, HOS).transpose(1, 0, 2).reshape(128, KT * HOS)
        )
        z = zeros[rows]
        z_h, z_l = _split_hi_lo(z)
        corrR = np.zeros((KC, OS), dtype=F16)
        corrR[0] = -128.0
        corrR[1] = -128.0
        corrR[2, :HOS] = -128.0
        corrR[3, :HOS] = -128.0
        corrR[4, HOS:] = -128.0
        corrR[5, HOS:] = -128.0
        corrR[6] = -z_h
        corrR[7] = -z_l
        corrR[8] = -z_h
        sc_c = np.broadcast_to(scales[rows].astype(np.float32), (B, OS)).copy()
        in_maps.append(
            {
                "q": q_dram,
                "stat": stat,
                "corrL": corrL,
                "corrR": corrR,
                "sc": sc_c,
            }
        )
    return in_maps


def kernel(inp, quant_weight, scales, zeros):
    from concourse.bass_utils import run_bass_kernel_spmd

    nc = _get_program()
    in_maps = _host_prep(inp, quant_weight, scales, zeros)
    res = run_bass_kernel_spmd(nc, in_maps, core_ids=list(range(NCORES)))
    out = np.concatenate(
        [res.results[c]["out"] for c in range(NCORES)], axis=1
    )
    return np.ascontiguousarray(out.astype(np.float32))


# revision 9
# speedup vs baseline: 1.8690x; 1.0811x over previous
"""4-bit column-block-quantized linear (ColBlockQuantizedLinear) on 8 TRN2 cores.

Math:  out[b,o] = scales[o] * (sum_i inp[b,i]*wq[o,i] - zeros[o]*rowsum[b])
with packed bytes q[j,o] (j = i//2): low nibble l = wq[o,2j], high nibble
h = wq[o,2j+1].  Identity: sum_j a_j*l_j + b_j*h_j = sum_j a_j*q_j + c_j*h_j
with c = b - 16a, q = 16h + l.

Device scheme (fp16 bit-trick): the fp16 bit pattern 0x5800|x encodes the
value 128 + x/8 EXACTLY for any 8-bit x.  So each weight stream is ONE
dual-op DVE tensor_scalar pass over the packed u16 data:
    Qlo = (q16 & 0x00FF) | 0x5800   -> 128 + q_lo/8      (pairs with 8a)
    Qhi = (q16 >>  8)    | 0x5800   -> 128 + q_hi/8      (pairs with 8a)
    Hlo = (q16 & 0x00F0) | 0x5800   -> 128 + 2*h_lo      (pairs with c/2)
    Hhi = (q16 >> 12)    | 0x5800   -> 128 + h_hi/8      (pairs with 8c)
The +128 offsets cancel exactly against rank-1 rows built from the SAME
fp16-rounded stationaries, folded with -zeros*rowsum into a K=9 fp16 hi/lo
correction matmul issued FIRST (start=True) so there is no tail.  Stationary
activation factors are single fp16.  Scales are applied on-device by
per-psum-block DVE tensor_tensor multiplies into one output tile.

A few zero matmuls on memset tiles warm the PE (HAM un-throttle needs
~3.4us of activity) while the first DMAs/DVE passes run.  kt tiles are
processed in chunks [1,1,2,2,...] - small first chunks cut pipeline rampup,
pairs amortize the ~150ns fixed DVE pass overhead.

Host byte layout: per core packed bytes [2048, 1376] are column-paired as
(m, 688+m) into uint16, rows regrouped so q_dram[r, kt*688+m] holds
contraction row kt*128+r -> one contiguous DMA per chunk.

Sharding: column-parallel over out_features (1376 rows/core), inputs
replicated; per-core output [16,1376] gathered on host.
"""

import numpy as np

B = 16
I = 4096
O = 11008
NCORES = 8
OS = O // NCORES          # 1376 out-features per core
HOS = OS // 2             # 688 packed u16 columns
HALF = I // 2             # 2048 packed (contraction) rows
KT = HALF // 128          # 16 contraction tiles
CHUNKS = [1, 1] + [2] * 7  # kt tiles per processing chunk
KC = 9                    # correction matmul contraction size
NDUMMY = 5                # PE warmup matmuls
# psum o-blocks (each within one 688-column half, <=512 cols per fp32 bank)
BLKS = [(0, 512), (512, 176), (688, 512), (1200, 176)]

F16 = np.float16

_CACHE = {}


def _build_program():
    import concourse.bacc as bacc
    import concourse.mybir as mybir
    import concourse.tile as tile

    dt = mybir.dt
    op = mybir.AluOpType
    nc = bacc.Bacc("TRN2", target_bir_lowering=False)

    q = nc.dram_tensor("q", [128, KT * HOS], dt.uint16, kind="ExternalInput")
    stat = nc.dram_tensor("stat", [128, KT * 48], dt.float16, kind="ExternalInput")
    corr = nc.dram_tensor("corr", [KC, 16 + OS], dt.float16, kind="ExternalInput")
    sc = nc.dram_tensor("sc", [B, OS], dt.float32, kind="ExternalInput")
    out = nc.dram_tensor("out", [B, OS], dt.float32, kind="ExternalOutput")

    cstart = [0]
    for w in CHUNKS:
        cstart.append(cstart[-1] + w)

    with tile.TileContext(nc) as tc:
        with (
            tc.tile_pool(name="consts", bufs=1) as cpool,
            tc.tile_pool(name="qp", bufs=3) as qpool,
            tc.tile_pool(name="wp", bufs=3) as wpool,
            tc.tile_pool(name="op", bufs=1) as opool,
            tc.tile_pool(name="ps", bufs=1, space="PSUM") as pspool,
        ):
            # PE warmup: zero matmuls while DMAs/DVE fill the pipeline
            dummy = cpool.tile([128, 512], dt.float16, name="dummy")
            ps_w = pspool.tile([16, 512], dt.float32, name="ps_w")
            nc.vector.memset(dummy, 0.0)
            for _ in range(NDUMMY):
                nc.tensor.matmul(
                    ps_w, dummy[:, 0:16], dummy, start=True, stop=True,
                    skip_group_check=True,
                )

            corr_sb = cpool.tile([KC, 16 + OS], dt.float16, name="corr_sb")
            stat_sb = cpool.tile([128, KT * 48], dt.float16, name="stat_sb")
            sc_sb = cpool.tile([B, OS], dt.float32, name="sc_sb")
            nc.scalar.dma_start(stat_sb, stat[:, :])
            nc.scalar.dma_start(corr_sb, corr[:, :])
            nc.scalar.dma_start(sc_sb, sc[:, :])
            corrL_sb = corr_sb[:, 0:16]
            corrR_sb = corr_sb[:, 16 : 16 + OS]

            psums = [
                pspool.tile([B, n], dt.float32, name=f"ps{i}")
                for i, (s, n) in enumerate(BLKS)
            ]

            for ci, cw in enumerate(CHUNKS):
                k0, w = cstart[ci], cw * HOS
                qt = qpool.tile([128, w], dt.uint16, name=f"qt{cw}", tag="qt")
                nc.sync.dma_start(qt, q[:, k0 * HOS : k0 * HOS + w])
                qlo = wpool.tile([128, w], dt.uint16, name=f"qlo{cw}", tag="qlo")
                qhi = wpool.tile([128, w], dt.uint16, name=f"qhi{cw}", tag="qhi")
                hlo = wpool.tile([128, w], dt.uint16, name=f"hlo{cw}", tag="hlo")
                hhi = wpool.tile([128, w], dt.uint16, name=f"hhi{cw}", tag="hhi")
                nc.vector.tensor_scalar(
                    qlo, qt, 0x00FF, 0x5800, op.bitwise_and, op.bitwise_or
                )
                nc.vector.tensor_scalar(
                    qhi, qt, 8, 0x5800, op.logical_shift_right, op.bitwise_or
                )
                nc.vector.tensor_scalar(
                    hlo, qt, 0x00F0, 0x5800, op.bitwise_and, op.bitwise_or
                )
                nc.vector.tensor_scalar(
                    hhi, qt, 12, 0x5800, op.logical_shift_right, op.bitwise_or
                )
                qlo16 = qlo.bitcast(dt.float16)
                qhi16 = qhi.bitcast(dt.float16)
                hlo16 = hlo.bitcast(dt.float16)
                hhi16 = hhi.bitcast(dt.float16)
                for h in range(cw):
                    kt = k0 + h
                    first = kt == 0
                    off = h * HOS
                    sq = stat_sb[:, kt * 48 : kt * 48 + 16]
                    shlo = stat_sb[:, kt * 48 + 16 : kt * 48 + 32]
                    shhi = stat_sb[:, kt * 48 + 32 : kt * 48 + 48]
                    for i, (s, n) in enumerate(BLKS):
                        if s < HOS:
                            a, b_ = off + s, off + s + n
                            nc.tensor.matmul(
                                psums[i], sq, qlo16[:, a:b_],
                                start=first, stop=False,
                            )
                            nc.tensor.matmul(
                                psums[i], shlo, hlo16[:, a:b_],
                                start=False, stop=False,
                            )
                        else:
                            a, b_ = off + s - HOS, off + s - HOS + n
                            nc.tensor.matmul(
                                psums[i], sq, qhi16[:, a:b_],
                                start=first, stop=False,
                            )
                            nc.tensor.matmul(
                                psums[i], shhi, hhi16[:, a:b_],
                                start=False, stop=False,
                            )

            # corrections last (PE is warm): -128*sum(coef) and -zeros*rowsum
            o = opool.tile([B, OS], dt.float32, name="o")
            for i, (s, n) in enumerate(BLKS):
                nc.tensor.matmul(
                    psums[i], corrL_sb, corrR_sb[:, s : s + n],
                    start=False, stop=True,
                )
                nc.vector.tensor_tensor(
                    o[:, s : s + n], psums[i], sc_sb[:, s : s + n], op.mult
                )
            nc.sync.dma_start(out[:, :], o)

    nc.finalize()
    return nc


def _get_program():
    if "nc" not in _CACHE:
        _CACHE["nc"] = _build_program()
    return _CACHE["nc"]


def _split_hi_lo(x64):
    hi = x64.astype(F16)
    lo = (x64 - hi.astype(np.float64)).astype(F16)
    return hi, lo


def _host_prep(inp, quant_weight, scales, zeros):
    """Per-core input maps: layout/precision prep only, no O(O*I) math."""
    inp64 = np.asarray(inp, dtype=np.float64)
    a = inp64[:, 0::2].T  # [HALF, B] even-i activations (pair with l / q)
    b = inp64[:, 1::2].T  # [HALF, B] odd-i activations (pair with h)
    c = b - 16.0 * a

    sq = (8.0 * a).astype(F16)      # [HALF, B]
    shlo = (c / 2.0).astype(F16)
    shhi = (8.0 * c).astype(F16)

    stat = np.zeros((128, KT * 48), dtype=F16)
    for kt in range(KT):
        rows = slice(kt * 128, (kt + 1) * 128)
        stat[:, kt * 48 : kt * 48 + 16] = sq[rows]
        stat[:, kt * 48 + 16 : kt * 48 + 32] = shlo[rows]
        stat[:, kt * 48 + 32 : kt * 48 + 48] = shhi[rows]

    # correction batch vectors from the ROUNDED stationaries (exact cancel)
    sum_sq = sq.astype(np.float64).sum(axis=0)      # [B]
    sum_shlo = shlo.astype(np.float64).sum(axis=0)
    sum_shhi = shhi.astype(np.float64).sum(axis=0)
    rowsum = inp64.sum(axis=1)                      # [B]
    sq_h, sq_l = _split_hi_lo(sum_sq)
    slo_h, slo_l = _split_hi_lo(sum_shlo)
    shi_h, shi_l = _split_hi_lo(sum_shhi)
    rs_h, rs_l = _split_hi_lo(rowsum)
    corrL = np.zeros((KC, 16), dtype=F16)
    corrL[0], corrL[1] = sq_h, sq_l
    corrL[2], corrL[3] = slo_h, slo_l
    corrL[4], corrL[5] = shi_h, shi_l
    corrL[6], corrL[7] = rs_h, rs_h
    corrL[8] = rs_l
    del sq_h, sq_l, slo_h, slo_l, shi_h, shi_l

    qw = np.asarray(quant_weight)
    scales = np.asarray(scales, dtype=np.float64).reshape(-1)
    zeros = np.asarray(zeros, dtype=np.float64).reshape(-1)

    in_maps = []
    for cidx in range(NCORES):
        rows = slice(cidx * OS, (cidx + 1) * OS)
        qc = qw[rows].astype(np.uint8).T  # [HALF, OS] natural columns
        # byte-pair columns (m, 688+m) -> uint16 elements
        qc2 = np.empty((HALF, OS), dtype=np.uint8)
        qc2[:, 0::2] = qc[:, :HOS]
        qc2[:, 1::2] = qc[:, HOS:]
        qu16 = np.ascontiguousarray(qc2).view(np.uint16)  # [HALF, HOS]
        # regroup rows: q_dram[r, kt*HOS + m] = qu16[kt*128 + r, m]
        q_dram = np.ascontiguousarray(
            qu16.reshape(KT, 128, HOS).transpose(1, 0, 2).reshape(128, KT * HOS)
        )
        z = zeros[rows]
        z_h, z_l = _split_hi_lo(z)
        corr_c = np.zeros((KC, 16 + OS), dtype=F16)
        corr_c[:, 0:16] = corrL
        corrR = corr_c[:, 16:]
        corrR[0] = -128.0
        corrR[1] = -128.0
        corrR[2, :HOS] = -128.0
        corrR[3, :HOS] = -128.0
        corrR[4, HOS:] = -128.0
        corrR[5, HOS:] = -128.0
        corrR[6] = -z_h
        corrR[7] = -z_l
        corrR[8] = -z_h
        sc_c = np.broadcast_to(scales[rows].astype(np.float32), (B, OS)).copy()
        in_maps.append(
            {
                "q": q_dram,
                "stat": stat,
                "corr": corr_c,
                "sc": sc_c,
            }
        )
    return in_maps


def kernel(inp, quant_weight, scales, zeros):
    from concourse.bass_utils import run_bass_kernel_spmd

    nc = _get_program()
    in_maps = _host_prep(inp, quant_weight, scales, zeros)
    res = run_bass_kernel_spmd(nc, in_maps, core_ids=list(range(NCORES)))
    out = np.concatenate(
        [res.results[c]["out"] for c in range(NCORES)], axis=1
    )
    return np.ascontiguousarray(out.astype(np.float32))
